# revision 7
# baseline (speedup 1.0000x reference)
"""Modulated 3x3 conv (StyleGAN2-style, no demodulation) on 8 TRN2 cores.

Algebraic restructuring: the style modulates the input-channel axis of the
weight, so conv(x, conv_scale * W * s[ci]) == conv(x * (conv_scale * s)[ci], W).
We therefore modulate the activations per channel on-device and run a single
shared-weight conv per sample. Data-parallel over batch: 1 sample per core.

Conv as 9 shifted matmuls accumulating in PSUM. The image lives in SBUF with
one zero column of horizontal padding on each side (66-px rows), so every tap
reads a full even-width 64-col window (fp32r ISA requires even element counts
and 8B-aligned PSUM addresses). Vertical boundaries are handled by clipping
the row range per tap: the center tap writes the full tile with start=True,
edge taps accumulate onto the already-written elements.

Matmuls run in float32r (TF32) — full PE rate with ~1e-3 relative accuracy.
"""

import math

import numpy as np

import concourse.bass as bass  # noqa: F401  (bass types referenced via bacc/tile)
import concourse.tile as tile
from concourse import bacc, mybir
from concourse.bass_utils import run_bass_kernel_spmd

B, CIN, COUT, KS, Z, H, W = 8, 512, 512, 3, 512, 64, 64
HW = H * W  # 4096
N_CORES = 8
CB = CIN // 128  # 4 input-channel blocks
OB = COUT // 128  # 4 output-channel blocks
NT = 8  # spatial tiles; each covers 8 rows of 64 px = 512 output positions
ROWS_PER_TILE = H // NT  # 8

F32 = mybir.dt.float32
F32R = mybir.dt.float32r

TAPS = [(dh, dw) for dh in (-1, 0, 1) for dw in (-1, 0, 1)]
# center tap first so the start=True matmul covers every PSUM element
TAP_ORDER = sorted(range(9), key=lambda t: (TAPS[t] != (0, 0)))

XW = W + 2  # 66: padded row width in SBUF
XSZ = H * XW  # 4224


def _tap_rows(nt: int, dh: int):
    """Valid output-row range for this tap within spatial tile nt."""
    h0 = nt * ROWS_PER_TILE
    h_lo = max(h0, -dh)
    h_hi = min(h0 + ROWS_PER_TILE - 1, H - 1 - dh)
    return h0, h_lo, h_hi


def build(loop_reps: int = 1):
    nc = bacc.Bacc("TRN2", target_bir_lowering=False, debug=False, num_devices=N_CORES)

    x_d = nc.dram_tensor("x", [CIN, HW], F32R, kind="ExternalInput").ap()
    w_d = nc.dram_tensor("w", [CB * 9 * 128, COUT], F32R, kind="ExternalInput").ap()
    s_d = nc.dram_tensor("s", [128, CB], F32, kind="ExternalInput").ap()
    o_d = nc.dram_tensor("out", [COUT, HW], F32, kind="ExternalOutput").ap()

    with tile.TileContext(nc) as tc:
        with (
            tc.tile_pool(name="xp", bufs=1) as xp,
            tc.tile_pool(name="xi", bufs=2) as xi,
            tc.tile_pool(name="wp", bufs=1) as wp,
            tc.tile_pool(name="sp", bufs=1) as sp,
            tc.tile_pool(name="op", bufs=4) as op,
            tc.tile_pool(name="ps", bufs=6, space="PSUM") as ps,
        ):

            def body(_i=None):
                st = sp.tile([128, CB], F32, name="st")
                nc.sync.dma_start(out=st[:], in_=s_d[:])

                xts = []
                for cb in range(CB):
                    xin = xi.tile([128, HW], F32R, name="xin")
                    nc.sync.dma_start(
                        out=xin[:], in_=x_d[cb * 128 : (cb + 1) * 128, :]
                    )
                    xt = xp.tile([128, XSZ], F32R, name=f"x{cb}")
                    # zero the horizontal padding columns: index 0, the
                    # adjacent (r*66+65, (r+1)*66) pairs, and the final 4223
                    x3p = xt[:].rearrange("p (h w) -> p h w", w=XW)
                    nc.vector.memset(xt[:, 0:1].bitcast(F32), 0.0)
                    pairs = xt[:, XW - 1 : XW - 1 + (H - 1) * XW].rearrange(
                        "p (r c) -> p r c", c=XW
                    )
                    nc.vector.memset(pairs[:, :, 0:2].bitcast(F32), 0.0)
                    nc.vector.memset(xt[:, XSZ - 1 : XSZ].bitcast(F32), 0.0)
                    # per-channel modulation (channel == partition), fused
                    # with relayout into the padded image
                    nc.vector.tensor_scalar_mul(
                        x3p[:, :, 1 : 1 + W],
                        xin[:].rearrange("p (h w) -> p h w", w=W),
                        st[:, cb : cb + 1],
                    )
                    xts.append(xt)

                wts = {}
                for cb in range(CB):
                    for t in TAP_ORDER:
                        wt = wp.tile([128, COUT], F32R, name=f"w{cb}_{t}")
                        r0 = (cb * 9 + t) * 128
                        nc.sync.dma_start(out=wt[:], in_=w_d[r0 : r0 + 128, :])
                        wts[cb, t] = wt

                for nt in range(NT):
                    for ob in range(OB):
                        pt = ps.tile([128, 512], F32, name="pt")
                        p3 = pt[:].rearrange("p (h w) -> p h w", w=W)
                        first = True
                        n_mm = len(TAP_ORDER) * CB
                        k = 0
                        for t in TAP_ORDER:
                            dh, dw = TAPS[t]
                            h0, h_lo, h_hi = _tap_rows(nt, dh)
                            out_ap = p3[:, h_lo - h0 : h_hi - h0 + 1, :]
                            for cb in range(CB):
                                x3 = xts[cb][:].rearrange(
                                    "p (h w) -> p h w", w=XW
                                )
                                rhs = x3[
                                    :,
                                    h_lo + dh : h_hi + dh + 1,
                                    1 + dw : 1 + dw + W,
                                ]
                                nc.tensor.matmul(
                                    out_ap,
                                    wts[cb, t][:, ob * 128 : (ob + 1) * 128],
                                    rhs,
                                    start=first,
                                    stop=(k == n_mm - 1),
                                )
                                first = False
                                k += 1
                        ot = op.tile([128, 512], F32, name="ot")
                        nc.vector.tensor_copy(ot[:], pt[:])
                        nc.sync.dma_start(
                            out=o_d[
                                ob * 128 : (ob + 1) * 128, nt * 512 : (nt + 1) * 512
                            ],
                            in_=ot[:],
                        )

            if loop_reps > 1:
                with tc.For_i(0, loop_reps, 1) as i:
                    body(i)
            else:
                body()

    nc.compile()
    return nc


_CACHE: dict[int, "bacc.Bacc"] = {}


def _get_nc(loop_reps: int = 1):
    if loop_reps not in _CACHE:
        _CACHE[loop_reps] = build(loop_reps)
    return _CACHE[loop_reps]


def _host_prep(input, style, weight, mod_w, mod_b):
    mod_scale = 1.0 / math.sqrt(Z)
    conv_scale = 1.0 / math.sqrt(CIN * KS * KS)
    s = style @ (mod_w * mod_scale).T + mod_b  # [B, CIN]
    s = (conv_scale * s).astype(np.float32)

    # wT[cb, t, ci128, co] = weight[0, co, cb*128+ci, kh, kw], t = kh*3+kw
    w0 = np.ascontiguousarray(weight[0])  # [co, ci, kh, kw]
    wt = w0.transpose(1, 2, 3, 0).reshape(CB, 128, 9, COUT)
    wt = np.ascontiguousarray(wt.transpose(0, 2, 1, 3)).reshape(CB * 9 * 128, COUT)
    wt = wt.astype(np.float32)
    return s, wt


def kernel(input, style, weight, mod_w, mod_b):
    input = np.asarray(input, dtype=np.float32)
    style = np.asarray(style, dtype=np.float32)
    weight = np.asarray(weight, dtype=np.float32)
    mod_w = np.asarray(mod_w, dtype=np.float32)
    mod_b = np.asarray(mod_b, dtype=np.float32)

    s, wt = _host_prep(input, style, weight, mod_w, mod_b)

    nc = _get_nc(1)
    in_maps = []
    for b in range(B):
        in_maps.append(
            {
                "x": np.ascontiguousarray(input[b].reshape(CIN, HW)),
                "w": wt,
                "s": np.ascontiguousarray(s[b].reshape(CB, 128).T),
            }
        )
    res = run_bass_kernel_spmd(nc, in_maps, list(range(N_CORES)))
    out = np.stack(
        [res.results[b]["out"].reshape(COUT, H, W) for b in range(B)], axis=0
    )
    return out
